# revision 67
# baseline (speedup 1.0000x reference)
"""CapsuleCONV Trainium2 kernel (nn_CapsuleCONV_1709396984016) — v3.

Math (per batch b):
  unfold input [N,32,32,16] with K=3,stride=2 -> patches X[n,k,l,hw,a,x]
  votes V[n,kl,hw,a,(d,m)] = sum_x X * w[k,l,n,x,d,m]
  logits qk[n,kl,m,hw] = 0.25 * sum_{a,d} V * ncv[b,m,hw,a,d]
  qk = softmax_m(qk);  out[m,hw,a,d] = sum_{n,kl} qk * V
  out = LayerNorm_{(a,d)}(out) * gamma + beta

Device mapping (8 cores, 4 batches each), per (bi, t=(kl,i)):
  input partitions p = 32i+4g+x (n = 8i+g); votes partitions
  p' = 16g+4j+c4 (j = pose col d, m = 4r+c4, r in free).
  votes: per (t, r): [32,128] lhsT at tile_position (32i,0), two 450-col
  matmuls (a-pairs) into a 2-bank PSUM tile, Act drain -> votes fp16.
  logits: a-sum on DVE (2 adds) -> ptsum [128,(r8,hw)]; SQ128 matmul
  per r-pair (sum j, replicate j); exp on Act (2x900).
  den: r-sum on DVE (3 adds) + one SD128 matmul (sum c4);
  reciprocal_approx_fast + fp16 cast; qkt = e*rd (r-bcast).
  p2 = votes*qkt (a-bcast); p2acc += p2 on DVE (fp16 ping-pong chain).
  out-agg: 16 SN matmuls per batch on p2acc (sum g, accumulate r).
  LayerNorm + output permute as selection matmuls; fp16 output.
"""
import numpy as np
from contextlib import ExitStack

import concourse.bass as bass
import concourse.tile as tile
from concourse import bacc, mybir
from concourse._compat import with_exitstack

F32 = mybir.dt.float32
F16 = mybir.dt.float16
NPF16 = np.float16

B, N, H, W, DIN = 32, 32, 32, 32, 16
M, DOUT = 32, 16
KK, STRIDE = 3, 2
HO = WO = 15
HWO = HO * WO  # 225
NCORES = 8
NB = B // NCORES  # 4 batches per core
LN_EPS = 1e-5
OHALVES = [(0, 113), (113, 112)]
KLS = [(k, l) for k in range(3) for l in range(3)]


# ---------------------------------------------------------------- host prep
def host_prep(input_, ncv, w, gamma, beta):
    # Contiguous unfold: xunf[b, 32i+4g+x, kl, ah, a2, hw] =
    #   input[b, 8i+g, k+2*hr, l+2*wr, 4*(2*ah+a2) + x]
    x6 = input_.reshape(B, N, H, W, 4, 4)        # [b, n, h, w, a, x]
    xT = x6.transpose(0, 1, 5, 2, 3, 4)          # [b, n, x, h, w, a]
    xT = xT.reshape(B, 128, H, W, 4)
    xunf = np.empty((B, 128, 9, 2, 2, HO, WO), np.float32)
    for kl, (k, l) in enumerate(KLS):
        patch = xT[:, :, k:k + 29:2, l:l + 29:2, :]   # [b, p, 15, 15, a]
        xunf[:, :, kl] = patch.transpose(0, 1, 4, 2, 3).reshape(
            B, 128, 2, 2, HO, WO)
    xunf = np.ascontiguousarray(xunf.reshape(B, 128, 9 * 4 * HWO))\
        .astype(NPF16)

    # Votes-side partition layout: p' = 16g + 4j + c4.
    # u_compact[b, 4j+c4, a, r, hw] = ncv[b, 4r+c4, hw, a*4+j]
    nc6 = ncv.reshape(B, 8, 4, HWO, 4, 4)        # [b, r, c4, hw, a, d]
    u = nc6.transpose(0, 5, 2, 4, 1, 3)          # [b, d(=j), c4, a, r, hw]
    u_scr = np.ascontiguousarray(
        u.reshape(B, 16, 4 * 8 * HWO)).astype(NPF16)

    # Compact weights: wc[kl, r, 32i+4g+x, 4j+c4] = w[k,l, 8i+g, x, j, 4r+c4].
    v = w.reshape(9, 4, 8, 4, 4, 8, 4)           # [kl, i, g, x, d(=j), r, c4]
    v = v.transpose(0, 5, 1, 2, 3, 4, 6)         # [kl, r, i, g, x, j, c4]
    w_c = np.ascontiguousarray(v.reshape(9, 8, 128, 16)).astype(NPF16)
    # gmask[(i,g,x), (g',j,c4)] = d_{g,g'}
    pp = np.arange(128)
    g_in = (pp % 32) // 4
    gmask = (g_in[:, None] == (np.arange(128) // 16)[None, :]).astype(NPF16)

    # SQ128[16g'+4j'+c', 16g+4j+c] = 0.25 d_{g,g'} d_{c,c'}  (sum j, rep j)
    jj = np.arange(128)
    g_row, j_row, c_row = jj // 16, (jj % 16) // 4, jj % 4
    sq = (0.25 * (g_row[:, None] == g_row[None, :])
          * (c_row[:, None] == c_row[None, :])).astype(np.float32)
    # SD128[p, p'] = d_{j,j'} d_{g,g'}   (sum over c4)
    sd = ((j_row[:, None] == j_row[None, :])
          * (g_row[:, None] == g_row[None, :])).astype(np.float32)
    # SN[r][(g,j,c4), 4*(4r+c4)+j] = 1   (sum over g)
    sn = np.zeros((8, 128, 128), np.float32)
    for r in range(8):
        for j in range(4):
            for g in range(8):
                for c in range(4):
                    sn[r, 16 * g + 4 * j + c, 4 * (4 * r + c) + j] = 1.0
    # SM[(4m+d), (4m'+d')] = d_{m,m'} / 16
    sm = np.zeros((128, 128), np.float32)
    for m in range(32):
        for d in range(4):
            for d2 in range(4):
                sm[4 * m + d, 4 * m + d2] = 1.0 / 16.0
    # Sd4[d][(4m+dd), m'] = d_{m,m'} d_{dd,d}
    sd4 = np.zeros((4, 128, 32), np.float32)
    for d in range(4):
        for m in range(32):
            sd4[d, 4 * m + d, m] = 1.0
    # per-partition gamma/beta columns: gcols[p, a] = gamma[a*4 + p%4]
    gcols = np.zeros((128, 4), np.float32)
    bcols = np.zeros((128, 4), np.float32)
    for p in range(128):
        for a in range(4):
            gcols[p, a] = gamma[a * 4 + p % 4]
            bcols[p, a] = beta[a * 4 + p % 4]

    return dict(xunf=xunf, u_scr=u_scr, w_c=w_c, gmask=gmask,
                sq=sq.astype(NPF16), sd=sd.astype(NPF16),
                sn=sn.astype(NPF16), sm=sm, sd4=sd4.astype(NPF16),
                gcols=gcols, bcols=bcols)


# ------------------------------------------------------------- tile program
@with_exitstack
def build_program(ctx: ExitStack, tc: tile.TileContext, dram: dict, nb=NB):
    nc = tc.nc

    const = ctx.enter_context(tc.tile_pool(name="const", bufs=1))
    xpool = ctx.enter_context(tc.tile_pool(name="xpool", bufs=1))
    upool = ctx.enter_context(tc.tile_pool(name="upool", bufs=2))
    vsb = ctx.enter_context(tc.tile_pool(name="vsb", bufs=3))
    ptp = ctx.enter_context(tc.tile_pool(name="ptp", bufs=1))
    psp = ctx.enter_context(tc.tile_pool(name="psp", bufs=2))  # s01 ring
    epool = ctx.enter_context(tc.tile_pool(name="epool", bufs=3))
    scr = ctx.enter_context(tc.tile_pool(name="scr", bufs=1))
    erp = ctx.enter_context(tc.tile_pool(name="erp", bufs=2))
    rdp = ctx.enter_context(tc.tile_pool(name="rdp", bufs=1))
    qktp = ctx.enter_context(tc.tile_pool(name="qktp", bufs=1))
    p2p = ctx.enter_context(tc.tile_pool(name="p2p", bufs=2))
    pap = ctx.enter_context(tc.tile_pool(name="pap", bufs=1))
    mpool = ctx.enter_context(tc.tile_pool(name="mpool", bufs=1))
    spool = ctx.enter_context(tc.tile_pool(name="spool", bufs=1))
    vps = ctx.enter_context(tc.tile_pool(name="vps", bufs=2, space="PSUM"))
    qps = ctx.enter_context(tc.tile_pool(name="qps", bufs=2, space="PSUM"))

    # constants. wc/gm are staged in the (idle at init) acc tile to avoid
    # dedicating SBUF to them.
    wbd_sb = const.tile([128, 9 * 8 * 128], F16, tag="wbd")
    scratch = pap.tile([128, 4 * 8 * HWO], F16, tag="pacc", name="initscr")
    wc_sb = scratch[:, :9 * 8 * 16]
    nc.sync.dma_start(
        wc_sb.rearrange("p (q r c) -> p q r c", q=9, r=8),
        dram["w_c"].rearrange("q r p c -> p q r c"))
    gm_sb = scratch[:, 2048:2048 + 128]
    nc.sync.dma_start(gm_sb, dram["gmask"])
    # expand block-diagonal w_bd = wc (bcast g') * gmask (bcast kl,r)
    nc.vector.tensor_mul(
        wbd_sb[:].rearrange("p (qr g jc) -> p qr g jc", qr=72, g=8),
        wc_sb.rearrange("p (qr one jc) -> p qr one jc", qr=72, one=1)
        .broadcast_to((128, 72, 8, 16)),
        gm_sb.rearrange("p (one g jc) -> p one g jc", one=1, g=8)
        .broadcast_to((128, 72, 8, 16)))
    sq_sb = const.tile([128, 128], F16, tag="sq")
    nc.sync.dma_start(sq_sb[:], dram["sq"])
    sd_sb = const.tile([128, 128], F16, tag="sd")
    nc.sync.dma_start(sd_sb[:], dram["sd"])
    sn_sb = const.tile([128, 8 * 128], F16, tag="sn")
    nc.sync.dma_start(sn_sb[:].rearrange("p (r c) -> p r c", r=8),
                      dram["sn"].rearrange("r p c -> p r c"))
    sm_sb = const.tile([128, 128], F32, tag="sm")
    nc.sync.dma_start(sm_sb[:], dram["sm"])
    sd4_sb = const.tile([128, 4 * 32], F16, tag="sd4")
    nc.sync.dma_start(sd4_sb[:].rearrange("p (d c) -> p d c", d=4),
                      dram["sd4"].rearrange("d p c -> p d c"))
    gc_sb = const.tile([128, 4], F32, tag="gc")
    nc.sync.dma_start(gc_sb[:], dram["gcols"])
    bc_sb = const.tile([128, 4], F32, tag="bc")
    nc.sync.dma_start(bc_sb[:], dram["bcols"])
    eps_sb = const.tile([128, 1], F32, tag="eps")
    nc.vector.memset(eps_sb[:], LN_EPS)

    wbd_v = wbd_sb[:].rearrange("p (q r c) -> p q r c", q=9, r=8)
    sn_v = sn_sb[:].rearrange("p (r c) -> p r c", r=8)
    sd4_v = sd4_sb[:].rearrange("p (d c) -> p d c", d=4)

    pending_tail = []

    for bi in range(nb):
        # split kl 0-4 / 5-8 so the next batch's first chunk can load as
        # soon as this batch's kl<5 votes are done (hides the reload)
        xuA = xpool.tile([128, 5 * 4 * HWO], F16, tag="xuA")
        nc.sync.dma_start(xuA[:], dram["xunf"][bi][:, :5 * 4 * HWO])
        xuB = xpool.tile([128, 4 * 4 * HWO], F16, tag="xuB")
        nc.sync.dma_start(xuB[:], dram["xunf"][bi][:, 5 * 4 * HWO:])
        xuA_v = xuA[:].rearrange("p (kl ah q) -> p kl ah q", kl=5, ah=2)
        xuB_v = xuB[:].rearrange("p (kl ah q) -> p kl ah q", kl=4, ah=2)

        def xu_v(kl, ah):
            return (xuA_v[:, kl, ah, :] if kl < 5
                    else xuB_v[:, kl - 5, ah, :])
        u_sb = upool.tile([128, 4 * 8 * HWO], F16, tag="u")
        for g in range(8):
            nc.sync.dma_start(u_sb[16 * g:16 * g + 16, :], dram["u_scr"][bi])

        # Software-pipelined over t = 4*kl + i (36 blocks/bi), 5 stages.
        T = 36
        st = {}

        def votes_mm(t, r, votes, drains):
            kl, i = divmod(t, 4)
            vt = vps.tile([128, 1024], F32, tag="vt", name=f"vt{t}_{r}")
            for ah in range(2):
                nc.tensor.matmul(
                    vt[:, 512 * ah:512 * ah + 450],
                    wbd_v[32 * i:32 * i + 32, kl, r, :],
                    xu_v(kl, ah)[32 * i:32 * i + 32, :],
                    start=True, stop=True,
                    tile_position=(32 * i, 0),
                )
            src = vt[:].rearrange("p (b q) -> p b q", b=2)[:, :, :450]\
                .rearrange("p b (a2 hw) -> p b a2 hw", a2=2)
            dst = votes[:].rearrange(
                "p (b a2 r hw) -> p b a2 r hw", b=2, a2=2, r=8)\
                [:, :, :, r, :]
            drains.append(('copy', dst, src))

        def emit_a(t):
            # pt = votes * u;  a-sum -> ptsum [128, (r8, hw)]
            votes = st['v', t]
            pt = ptp.tile([128, 4 * 8 * HWO], F16, tag="pt", name=f"pt{t}")
            nc.vector.tensor_mul(pt[:], votes[:], u_sb[:])
            s01 = psp.tile([128, 2 * 8 * HWO], F16, tag="s01",
                           name=f"s01_{t}")
            nc.vector.tensor_add(s01[:], pt[:][:, :2 * 8 * HWO],
                                 pt[:][:, 2 * 8 * HWO:])
            st['s01', t] = s01

        def sq_mm(t, half, exps):
            # logits (sum j via SQ, second a-fold via PSUM accum)
            s01 = st['s01', t]
            pv = s01[:].rearrange("p (a2 r hw) -> p a2 r hw", a2=2, r=8)
            e_sb = st['e', t]
            qk = qps.tile([128, 1024], F32, tag="qk",
                          name=f"qk{t}_{half}")
            for b2 in range(2):
                r2 = 2 * half + b2
                for a2 in range(2):
                    nc.tensor.matmul(
                        qk[:, 512 * b2:512 * b2 + 2 * HWO], sq_sb[:],
                        pv[:, a2, 2 * r2:2 * r2 + 2, :],
                        start=(a2 == 0), stop=(a2 == 1))
            dst = e_sb[:, half * 4 * HWO:(half + 1) * 4 * HWO]\
                .rearrange("p (b q) -> p b q", b=2)
            srcv = qk[:].rearrange("p (b q) -> p b q", b=2)[:, :, :2 * HWO]
            exps.append(('exp', dst, srcv))

        def emit_ers(t):
            # r-sum of e on DVE -> er (feeds next step's den matmul)
            e_sb = st['e', t]
            s1 = scr.tile([128, 4 * HWO], F16, tag="ers1", name=f"ers1_{t}")
            nc.vector.tensor_add(s1[:], e_sb[:][:, :4 * HWO],
                                 e_sb[:][:, 4 * HWO:])
            s2 = scr.tile([128, 2 * HWO], F16, tag="ers2", name=f"ers2_{t}")
            nc.vector.tensor_add(s2[:], s1[:][:, :2 * HWO],
                                 s1[:][:, 2 * HWO:])
            er = erp.tile([128, HWO], F16, tag="er", name=f"er{t}")
            nc.vector.tensor_add(er[:], s2[:][:, :HWO], s2[:][:, HWO:])
            st['er', t] = er

        def den_mm(t):
            er = st.pop(('er', t))
            den = qps.tile([128, 1024], F32, tag="qk", name=f"den{t}")
            nc.tensor.matmul(den[:, :HWO], sd_sb[:], er[:],
                             start=True, stop=True)
            st['den', t] = den

        def emit_qkt(t):
            # rd = 1/den; qkt = e * rd (r-bcast)
            e_sb = st.pop(('e', t))
            den = st.pop(('den', t))
            rd = rdp.tile([128, HWO], F32, tag="rd", name=f"rd{t}")
            nc.vector.reciprocal_approx_fast(out=rd[:], in_=den[:, :HWO])
            rdh = rdp.tile([128, HWO], F16, tag="rdh", name=f"rdh{t}")
            nc.vector.tensor_copy(rdh[:], rd[:])
            e_v = e_sb[:].rearrange("p (r hw) -> p r hw", r=8)
            qkt = qktp.tile([128, 8 * HWO], F16, tag="qkt", name=f"qkt{t}")
            rdb = rdh[:].rearrange("p (o hw) -> p o hw", o=1)\
                .broadcast_to((128, 8, HWO))
            nc.vector.tensor_mul(
                qkt[:].rearrange("p (r hw) -> p r hw", r=8), e_v, rdb)
            st['qkt', t] = qkt

        def emit_d(t):
            # p2 = votes * qkt (a-bcast); acc += p2 (gpsimd RMW-accum DMA)
            votes = st.pop(('v', t))
            qkt = st.pop(('qkt', t))
            qb = qkt[:].rearrange("p (o q) -> p o q", o=1)\
                .broadcast_to((128, 4, 8 * HWO))
            Q = 8 * HWO
            if t == 0:
                acc = pap.tile([128, 4 * Q], F16, tag="pacc",
                               name=f"pacc{bi}")
                dst = acc
                st['acc'] = acc
            else:
                dst = p2p.tile([128, 4 * Q], F16, tag="p2", name=f"p2_{t}")
            # a=3 slice on the (otherwise idle) gpsimd engine, rest on DVE
            nc.gpsimd.tensor_mul(
                dst[:].rearrange("p (a q) -> p a q", a=4)[:, 3:4, :],
                votes[:].rearrange("p (a q) -> p a q", a=4)[:, 3:4, :],
                qb[:, 0:1, :])
            nc.vector.tensor_mul(
                dst[:].rearrange("p (a q) -> p a q", a=4)[:, :3, :],
                votes[:].rearrange("p (a q) -> p a q", a=4)[:, :3, :],
                qb[:, 0:1, :].broadcast_to((128, 3, Q)))
            if t > 0:
                acc = st['acc']
                for c in range(0, 4 * Q, Q):
                    nc.gpsimd.dma_start(
                        acc[:][:, c:c + Q], dst[:][:, c:c + Q],
                        accum_op=mybir.AluOpType.add)

        # Per step s the stages are: V(s) votes, A(s-1) pt/s01,
        # B(s-2) SQ+exp+ers, C(s-3) den/rd/qkt, D(s-4) p2/acc.
        # PE work is interleaved (den first, SQ between votes) so the
        # tensor engine streams without ring-stall gaps (HAM stays warm);
        # Act instructions are ordered by producer completion.
        for step in range(T + 4):
            if pending_tail:
                pending_tail.pop(0)()
            # --- PE head: den for t=s-3 (er was finished last step)
            if 3 <= step <= T + 2:
                den_mm(step - 3)
            # --- DVE: p2 first (inputs ready), then pt/s01 (ready), then
            # the rd/qkt chain (waits on this step's den matmul)
            if 4 <= step <= T + 3:
                emit_d(step - 4)
            if 1 <= step <= T:
                emit_a(step - 1)
            if 3 <= step <= T + 2:
                emit_qkt(step - 3)
            # --- PE stream (+ matching Act order)
            tb = step - 2 if 2 <= step <= T + 1 else None
            drains, exps = [], []
            if tb is not None:
                st['e', tb] = epool.tile([128, 8 * HWO], F16, tag="e",
                                         name=f"e{tb}")
            if tb is not None:
                sq_mm(tb, 0, exps)
                sq_mm(tb, 1, exps)
            if step < T:
                votes = vsb.tile([128, 4 * 8 * HWO], F16, tag="votes",
                                 name=f"votes{bi}_{step}")
                for r in range(8):
                    votes_mm(step, r, votes, drains)
                st['v', step] = votes
            # --- Act: exps first (feed this step's DVE r-sum), then drains
            order = exps + drains
            for kind, dst, src in order:
                if kind == 'exp':
                    nc.scalar.activation(
                        dst, src, mybir.ActivationFunctionType.Exp)
                else:
                    nc.scalar.copy(dst, src)
            # --- DVE tail of step: r-sum of this step's e
            if tb is not None:
                emit_ers(tb)

        pending_tail = _make_tail(tc, bi, st.pop('acc'), dram,
                                  consts=dict(sn_v=sn_v, sm_sb=sm_sb,
                                              sd4_v=sd4_v, gc_sb=gc_sb,
                                              bc_sb=bc_sb, eps_sb=eps_sb),
                                  pools=dict(mpool=mpool, spool=spool,
                                             qps=qps))

    for chunk in pending_tail:
        chunk()


def _make_tail(tc, bi, acc, dram, consts, pools):
    """LayerNorm + output permute for batch bi, split into chunks that are
    emitted interleaved with the next batch's pipeline steps."""
    nc = tc.nc
    sn_v, sm_sb, sd4_v = consts["sn_v"], consts["sm_sb"], consts["sd4_v"]
    gc_sb, bc_sb, eps_sb = consts["gc_sb"], consts["bc_sb"], consts["eps_sb"]
    mpool, spool, qps = pools["mpool"], pools["spool"], pools["qps"]
    env = {}

    def c0():
        # out-agg: 16 SN matmuls on p2acc; drain to SBUF right away
        acc_v = acc[:].rearrange("p (a r hw) -> p a r hw", a=4, r=8)
        oacc_t = qps.tile([128, 1024], F32, tag="qk", name=f"oacc{bi}")
        for r in range(8):
            for h in range(2):
                o0, onh = OHALVES[h]
                nc.tensor.matmul(
                    oacc_t[:, 512 * h:512 * h + 4 * onh], sn_v[:, r, :],
                    acc_v[:, :, r, o0:o0 + onh],
                    start=(r == 0), stop=(r == 7),
                )
        oaccS = spool.tile([128, 4 * HWO], F16, tag="oaccS")
        for h in range(2):
            o0, onh = OHALVES[h]
            nc.scalar.copy(
                oaccS[:, 4 * o0:4 * o0 + 4 * onh],
                oacc_t[:, 512 * h:512 * h + 4 * onh])
        env["oaccS"] = oaccS

    def c1():
        oaccS = env["oaccS"]
        s1 = mpool.tile([128, HWO], F32, tag="s1")
        s2 = mpool.tile([128, HWO], F32, tag="s2")
        sqt = spool.tile([128, 4 * HWO], F16, tag="outn",
                         name=f"sqt{bi}")
        for h in range(2):
            o0, onh = OHALVES[h]
            oa = oaccS[:, 4 * o0:4 * o0 + 4 * onh]
            nc.vector.tensor_reduce(
                s1[:, o0:o0 + onh],
                oa.rearrange("p (a q) -> p q a", a=4),
                axis=mybir.AxisListType.X, op=mybir.AluOpType.add)
            nc.scalar.activation(sqt[:, 4 * o0:4 * o0 + 4 * onh], oa,
                                 mybir.ActivationFunctionType.Square)
            nc.vector.tensor_reduce(
                s2[:, o0:o0 + onh],
                sqt[:, 4 * o0:4 * o0 + 4 * onh]
                .rearrange("p (a q) -> p q a", a=4),
                axis=mybir.AxisListType.X, op=mybir.AluOpType.add)
        mu_t = qps.tile([128, 1024], F32, tag="qk", name=f"mu{bi}")
        nc.tensor.matmul(mu_t[:, :HWO], sm_sb[:], s1[:],
                         start=True, stop=True)
        nc.tensor.matmul(mu_t[:, 512:512 + HWO], sm_sb[:], s2[:],
                         start=True, stop=True)
        mu = mpool.tile([128, HWO], F32, tag="mu")
        nc.scalar.copy(mu[:], mu_t[:, :HWO])
        # reuse s1/s2 buffers: musq -> s1, var -> s2
        nc.vector.tensor_mul(s1[:], mu[:], mu[:])
        nc.vector.tensor_sub(s2[:], mu_t[:, 512:512 + HWO], s1[:])
        env["mu"], env["s1"], env["var"] = mu, s1, s2

    def c2():
        mu, var, sig = env["mu"], env["var"], env["s1"]
        oaccS = env["oaccS"]
        nc.scalar.activation(sig[:], var[:],
                             mybir.ActivationFunctionType.Sqrt,
                             bias=eps_sb[:, 0:1])
        rstd = var  # reuse
        nc.vector.reciprocal(rstd[:], sig[:])
        outn = spool.tile([128, 4 * HWO], F16, tag="outn")  # (a, hw)
        t3 = mpool.tile([128, HWO], F32, tag="t3")
        for h in range(2):
            o0, onh = OHALVES[h]
            for a in range(4):
                nc.vector.tensor_sub(
                    t3[:, :onh],
                    oaccS[:, 4 * o0 + a * onh:4 * o0 + (a + 1) * onh],
                    mu[:, o0:o0 + onh])
                nc.vector.tensor_mul(t3[:, :onh], t3[:, :onh],
                                     rstd[:, o0:o0 + onh])
                nc.vector.tensor_scalar(
                    outn[:, a * HWO + o0:a * HWO + o0 + onh],
                    t3[:, :onh], gc_sb[:, a:a + 1], bc_sb[:, a:a + 1],
                    op0=mybir.AluOpType.mult, op1=mybir.AluOpType.add)
        env["outn"] = outn

    def c3():
        outn = env["outn"]
        outn_v = outn[:].rearrange("p (a hw) -> p a hw", a=4)
        out_v = dram["out"][bi].rearrange("p (hw ad) -> p hw ad", ad=16)
        for h in range(2):
            o0, onh = OHALVES[h]
            fin = spool.tile([32, 113 * 16], F16, tag="fin",
                             name=f"fin{bi}_{h}")
            fin_v = fin[:][:, :onh * 16].rearrange(
                "p (hw a x) -> p a hw x", a=4, x=4)
            for d in range(4):
                fp_t = qps.tile([128, 1024], F32, tag="qk",
                                name=f"fp{bi}_{h}_{d}")
                fp = fp_t[:32, :4 * onh]
                nc.tensor.matmul(
                    fp, sd4_v[:, d, :],
                    outn_v[:, :, o0:o0 + onh],
                    start=True, stop=True)
                nc.scalar.copy(fin_v[:, :, :, d],
                               fp.rearrange("p (a q) -> p a q", a=4))
            nc.sync.dma_start(out_v[:, o0:o0 + onh, :],
                              fin[:][:, :onh * 16])

    return [c0, c1, c2, c3]


# ------------------------------------------------------------------ driver
def _build_nc(nb=NB):
    nc = bacc.Bacc("TRN2", target_bir_lowering=False, debug=False,
                   num_devices=NCORES)
    dram = {}
    dram["xunf"] = nc.dram_tensor("xunf", (nb, 128, 9 * 4 * HWO), F16,
                                  kind="ExternalInput").ap()
    dram["u_scr"] = nc.dram_tensor("u_scr", (nb, 16, 4 * 8 * HWO), F16,
                                   kind="ExternalInput").ap()
    dram["w_c"] = nc.dram_tensor("w_c", (9, 8, 128, 16), F16,
                                 kind="ExternalInput").ap()
    dram["gmask"] = nc.dram_tensor("gmask", (128, 128), F16,
                                   kind="ExternalInput").ap()
    dram["sq"] = nc.dram_tensor("sq", (128, 128), F16,
                                kind="ExternalInput").ap()
    dram["sd"] = nc.dram_tensor("sd", (128, 128), F16,
                                kind="ExternalInput").ap()
    dram["sn"] = nc.dram_tensor("sn", (8, 128, 128), F16,
                                kind="ExternalInput").ap()
    dram["sm"] = nc.dram_tensor("sm", (128, 128), F32,
                                kind="ExternalInput").ap()
    dram["sd4"] = nc.dram_tensor("sd4", (4, 128, 32), F16,
                                 kind="ExternalInput").ap()
    dram["gcols"] = nc.dram_tensor("gcols", (128, 4), F32,
                                   kind="ExternalInput").ap()
    dram["bcols"] = nc.dram_tensor("bcols", (128, 4), F32,
                                   kind="ExternalInput").ap()
    dram["out"] = nc.dram_tensor("out", (nb, 32, HWO * 16), F16,
                                 kind="ExternalOutput").ap()
    with tile.TileContext(nc) as tc:
        build_program(tc, dram, nb=nb)
    nc.compile()
    return nc


def _make_in_maps(hp):
    shared = {k: hp[k] for k in
              ("w_c", "gmask", "sq", "sd", "sn", "sm", "sd4",
               "gcols", "bcols")}
    in_maps = []
    for c in range(NCORES):
        im = dict(shared)
        im["xunf"] = np.ascontiguousarray(hp["xunf"][c * NB:(c + 1) * NB])
        im["u_scr"] = np.ascontiguousarray(hp["u_scr"][c * NB:(c + 1) * NB])
        in_maps.append(im)
    return in_maps


def _prep(**inputs):
    input_ = np.asarray(inputs["input"], dtype=np.float32)
    ncv = np.asarray(inputs["next_capsule_value"], dtype=np.float32)
    w = np.asarray(inputs["w"], dtype=np.float32)
    gamma = np.asarray(inputs["gamma"], dtype=np.float32)
    beta = np.asarray(inputs["beta"], dtype=np.float32)
    return host_prep(input_, ncv, w, gamma, beta)


def _to_full(outs):
    full = np.concatenate([np.asarray(o["out"]) for o in outs], axis=0)
    return full.reshape(B, M, HO, WO, DOUT).astype(np.float32)


def kernel(**inputs):
    hp = _prep(**inputs)
    nc = _build_nc()
    in_maps = _make_in_maps(hp)
    from concourse.bass_utils import run_bass_kernel_spmd
    res = run_bass_kernel_spmd(nc, in_maps, core_ids=list(range(NCORES)),
                               trace=False)
    return _to_full(res.results)


def kernel_traced(**inputs):
    hp = _prep(**inputs)
    nc = _build_nc()
    in_maps = _make_in_maps(hp)
    from concourse.bass_utils import run_bass_kernel_spmd
    res = run_bass_kernel_spmd(nc, in_maps, core_ids=list(range(NCORES)),
                               trace=True)
    return _to_full(res.results), res


def kernel_bench(reps=30, **inputs):
    """Correctness output + repeated-call wall times (ns) of the compiled
    8-core executable (device-resident inputs)."""
    import time
    import jax
    from jax.experimental.shard_map import shard_map
    from jax.sharding import Mesh, PartitionSpec
    from concourse import bass2jax, mybir as _mb

    hp = _prep(**inputs)
    nc = _build_nc()
    in_maps = _make_in_maps(hp)

    bass2jax.install_neuronx_cc_hook()
    partition_name = (nc.partition_id_tensor.name
                      if nc.partition_id_tensor else None)
    in_names, out_names, out_avals, zero_outs = [], [], [], []
    for alloc in nc.m.functions[0].allocations:
        if not isinstance(alloc, _mb.MemoryLocationSet):
            continue
        name = alloc.memorylocations[0].name
        if alloc.kind == "ExternalInput":
            if name != partition_name:
                in_names.append(name)
        elif alloc.kind == "ExternalOutput":
            shape = tuple(alloc.tensor_shape)
            dtype = _mb.dt.np(alloc.dtype)
            out_names.append(name)
            out_avals.append(jax.core.ShapedArray(shape, dtype))
            zero_outs.append(np.zeros(shape, dtype))
    n_params = len(in_names)
    all_in_names = list(in_names) + list(out_names)
    if partition_name is not None:
        all_in_names.append(partition_name)

    def _body(*args):
        operands = list(args)
        if partition_name is not None:
            operands.append(bass2jax.partition_id_tensor())
        outs = bass2jax._bass_exec_p.bind(
            *operands, out_avals=tuple(out_avals),
            in_names=tuple(all_in_names), out_names=tuple(out_names),
            lowering_input_output_aliases=(),
            sim_require_finite=True, sim_require_nnan=True, nc=nc)
        return tuple(outs)

    devices = jax.devices()[:NCORES]
    mesh = Mesh(np.asarray(devices), ("core",))
    n_outs = len(out_names)
    sharded = jax.jit(
        shard_map(_body, mesh=mesh,
                  in_specs=(PartitionSpec("core"),) * (n_params + n_outs),
                  out_specs=(PartitionSpec("core"),) * n_outs,
                  check_rep=False),
        keep_unused=True)
    concat_in = [np.concatenate([np.asarray(in_maps[c][nm])
                                 for c in range(NCORES)], axis=0)
                 for nm in in_names]
    concat_zeros = [np.zeros((NCORES * z.shape[0], *z.shape[1:]), z.dtype)
                    for z in zero_outs]
    dev_in = [jax.device_put(a) for a in concat_in]
    dev_zero = [jax.device_put(a) for a in concat_zeros]
    outs = sharded(*dev_in, *dev_zero)
    jax.block_until_ready(outs)
    ts = []
    for _ in range(reps):
        t0 = time.perf_counter()
        o = sharded(*dev_in, *dev_zero)
        jax.block_until_ready(o)
        ts.append(time.perf_counter() - t0)
    out_full = np.asarray(outs[out_names.index("out")]).reshape(
        NCORES, NB, 32, HWO * 16)
    full = out_full.reshape(B, M, HO, WO, DOUT).astype(np.float32)
    return full, np.array(ts) * 1e9


if __name__ == "__main__":
    nc = _build_nc()
    print("built OK")
    from concourse.timeline_sim import TimelineSim
    ts = TimelineSim(nc, no_exec=True)
    print("TimelineSim duration ns:", ts.simulate())


# revision 68
# speedup vs baseline: 1.0748x; 1.0748x over previous
"""CapsuleCONV Trainium2 kernel (nn_CapsuleCONV_1709396984016) — v3.

Math (per batch b):
  unfold input [N,32,32,16] with K=3,stride=2 -> patches X[n,k,l,hw,a,x]
  votes V[n,kl,hw,a,(d,m)] = sum_x X * w[k,l,n,x,d,m]
  logits qk[n,kl,m,hw] = 0.25 * sum_{a,d} V * ncv[b,m,hw,a,d]
  qk = softmax_m(qk);  out[m,hw,a,d] = sum_{n,kl} qk * V
  out = LayerNorm_{(a,d)}(out) * gamma + beta

Device mapping (8 cores, 4 batches each), per (bi, t=(kl,i)):
  input partitions p = 32i+4g+x (n = 8i+g); votes partitions
  p' = 16g+4j+c4 (j = pose col d, m = 4r+c4, r in free).
  votes: per (t, r): [32,128] lhsT at tile_position (32i,0), two 450-col
  matmuls (a-pairs) into a 2-bank PSUM tile, Act drain -> votes fp16.
  logits: a-sum on DVE (2 adds) -> ptsum [128,(r8,hw)]; SQ128 matmul
  per r-pair (sum j, replicate j); exp on Act (2x900).
  den: r-sum on DVE (3 adds) + one SD128 matmul (sum c4);
  reciprocal_approx_fast + fp16 cast; qkt = e*rd (r-bcast).
  p2 = votes*qkt (a-bcast); p2acc += p2 on DVE (fp16 ping-pong chain).
  out-agg: 16 SN matmuls per batch on p2acc (sum g, accumulate r).
  LayerNorm + output permute as selection matmuls; fp16 output.
"""
import numpy as np
from contextlib import ExitStack

import concourse.bass as bass
import concourse.tile as tile
from concourse import bacc, mybir
from concourse._compat import with_exitstack

F32 = mybir.dt.float32
F16 = mybir.dt.float16
NPF16 = np.float16

B, N, H, W, DIN = 32, 32, 32, 32, 16
M, DOUT = 32, 16
KK, STRIDE = 3, 2
HO = WO = 15
HWO = HO * WO  # 225
NCORES = 8
NB = B // NCORES  # 4 batches per core
LN_EPS = 1e-5
OHALVES = [(0, 113), (113, 112)]
KLS = [(k, l) for k in range(3) for l in range(3)]


# ---------------------------------------------------------------- host prep
def host_prep(input_, ncv, w, gamma, beta):
    # Contiguous unfold: xunf[b, 32i+4g+x, kl, ah, a2, hw] =
    #   input[b, 8i+g, k+2*hr, l+2*wr, 4*(2*ah+a2) + x]
    x6 = input_.reshape(B, N, H, W, 4, 4)        # [b, n, h, w, a, x]
    xT = x6.transpose(0, 1, 5, 2, 3, 4)          # [b, n, x, h, w, a]
    xT = xT.reshape(B, 128, H, W, 4)
    xunf = np.empty((B, 128, 9, 2, 2, HO, WO), np.float32)
    for kl, (k, l) in enumerate(KLS):
        patch = xT[:, :, k:k + 29:2, l:l + 29:2, :]   # [b, p, 15, 15, a]
        xunf[:, :, kl] = patch.transpose(0, 1, 4, 2, 3).reshape(
            B, 128, 2, 2, HO, WO)
    xunf = np.ascontiguousarray(xunf.reshape(B, 128, 9 * 4 * HWO))\
        .astype(NPF16)

    # Votes-side partition layout: p' = 16g + 4j + c4.
    # u_compact[b, 4j+c4, a, r, hw] = ncv[b, 4r+c4, hw, a*4+j]
    nc6 = ncv.reshape(B, 8, 4, HWO, 4, 4)        # [b, r, c4, hw, a, d]
    u = nc6.transpose(0, 5, 2, 4, 1, 3)          # [b, d(=j), c4, a, r, hw]
    u_scr = np.ascontiguousarray(
        u.reshape(B, 16, 4 * 8 * HWO)).astype(NPF16)

    # Compact weights: wc[kl, r, 32i+4g+x, 4j+c4] = w[k,l, 8i+g, x, j, 4r+c4].
    v = w.reshape(9, 4, 8, 4, 4, 8, 4)           # [kl, i, g, x, d(=j), r, c4]
    v = v.transpose(0, 5, 1, 2, 3, 4, 6)         # [kl, r, i, g, x, j, c4]
    w_c = np.ascontiguousarray(v.reshape(9, 8, 128, 16)).astype(NPF16)
    # gmask[(i,g,x), (g',j,c4)] = d_{g,g'}
    pp = np.arange(128)
    g_in = (pp % 32) // 4
    gmask = (g_in[:, None] == (np.arange(128) // 16)[None, :]).astype(NPF16)

    # SQ128[16g'+4j'+c', 16g+4j+c] = 0.25 d_{g,g'} d_{c,c'}  (sum j, rep j)
    jj = np.arange(128)
    g_row, j_row, c_row = jj // 16, (jj % 16) // 4, jj % 4
    sq = (0.25 * (g_row[:, None] == g_row[None, :])
          * (c_row[:, None] == c_row[None, :])).astype(np.float32)
    # SD128[p, p'] = d_{j,j'} d_{g,g'}   (sum over c4)
    sd = ((j_row[:, None] == j_row[None, :])
          * (g_row[:, None] == g_row[None, :])).astype(np.float32)
    # SN[r][(g,j,c4), 4*(4r+c4)+j] = 1   (sum over g)
    sn = np.zeros((8, 128, 128), np.float32)
    for r in range(8):
        for j in range(4):
            for g in range(8):
                for c in range(4):
                    sn[r, 16 * g + 4 * j + c, 4 * (4 * r + c) + j] = 1.0
    # SM[(4m+d), (4m'+d')] = d_{m,m'} / 16
    sm = np.zeros((128, 128), np.float32)
    for m in range(32):
        for d in range(4):
            for d2 in range(4):
                sm[4 * m + d, 4 * m + d2] = 1.0 / 16.0
    # Sd4[d][(4m+dd), m'] = d_{m,m'} d_{dd,d}
    sd4 = np.zeros((4, 128, 32), np.float32)
    for d in range(4):
        for m in range(32):
            sd4[d, 4 * m + d, m] = 1.0
    # per-partition gamma/beta columns: gcols[p, a] = gamma[a*4 + p%4]
    gcols = np.zeros((128, 4), np.float32)
    bcols = np.zeros((128, 4), np.float32)
    for p in range(128):
        for a in range(4):
            gcols[p, a] = gamma[a * 4 + p % 4]
            bcols[p, a] = beta[a * 4 + p % 4]

    return dict(xunf=xunf, u_scr=u_scr, w_c=w_c, gmask=gmask,
                sq=sq.astype(NPF16), sd=sd.astype(NPF16),
                sn=sn.astype(NPF16), sm=sm, sd4=sd4.astype(NPF16),
                gcols=gcols, bcols=bcols)


# ------------------------------------------------------------- tile program
@with_exitstack
def build_program(ctx: ExitStack, tc: tile.TileContext, dram: dict, nb=NB):
    nc = tc.nc

    const = ctx.enter_context(tc.tile_pool(name="const", bufs=1))
    xpool = ctx.enter_context(tc.tile_pool(name="xpool", bufs=1))
    upool = ctx.enter_context(tc.tile_pool(name="upool", bufs=2))
    vsb = ctx.enter_context(tc.tile_pool(name="vsb", bufs=3))
    ptp = ctx.enter_context(tc.tile_pool(name="ptp", bufs=1))
    psp = ctx.enter_context(tc.tile_pool(name="psp", bufs=2))  # s01 ring
    epool = ctx.enter_context(tc.tile_pool(name="epool", bufs=3))
    scr = ctx.enter_context(tc.tile_pool(name="scr", bufs=1))
    erp = ctx.enter_context(tc.tile_pool(name="erp", bufs=2))
    rdp = ctx.enter_context(tc.tile_pool(name="rdp", bufs=1))
    qktp = ctx.enter_context(tc.tile_pool(name="qktp", bufs=1))
    p2p = ctx.enter_context(tc.tile_pool(name="p2p", bufs=2))
    pap = ctx.enter_context(tc.tile_pool(name="pap", bufs=1))
    mpool = ctx.enter_context(tc.tile_pool(name="mpool", bufs=1))
    spool = ctx.enter_context(tc.tile_pool(name="spool", bufs=1))
    vps = ctx.enter_context(tc.tile_pool(name="vps", bufs=2, space="PSUM"))
    qps = ctx.enter_context(tc.tile_pool(name="qps", bufs=2, space="PSUM"))

    # constants. wc/gm are staged in the (idle at init) acc tile to avoid
    # dedicating SBUF to them.
    wbd_sb = const.tile([128, 9 * 8 * 128], F16, tag="wbd")
    scratch = pap.tile([128, 4 * 8 * HWO], F16, tag="pacc", name="initscr")
    wc_sb = scratch[:, :9 * 8 * 16]
    nc.sync.dma_start(
        wc_sb.rearrange("p (q r c) -> p q r c", q=9, r=8),
        dram["w_c"].rearrange("q r p c -> p q r c"))
    gm_sb = scratch[:, 2048:2048 + 128]
    nc.sync.dma_start(gm_sb, dram["gmask"])
    # expand block-diagonal w_bd = wc (bcast g') * gmask (bcast kl,r)
    nc.vector.tensor_mul(
        wbd_sb[:].rearrange("p (qr g jc) -> p qr g jc", qr=72, g=8),
        wc_sb.rearrange("p (qr one jc) -> p qr one jc", qr=72, one=1)
        .broadcast_to((128, 72, 8, 16)),
        gm_sb.rearrange("p (one g jc) -> p one g jc", one=1, g=8)
        .broadcast_to((128, 72, 8, 16)))
    sq_sb = const.tile([128, 128], F16, tag="sq")
    nc.sync.dma_start(sq_sb[:], dram["sq"])
    sd_sb = const.tile([128, 128], F16, tag="sd")
    nc.sync.dma_start(sd_sb[:], dram["sd"])
    sn_sb = const.tile([128, 8 * 128], F16, tag="sn")
    nc.sync.dma_start(sn_sb[:].rearrange("p (r c) -> p r c", r=8),
                      dram["sn"].rearrange("r p c -> p r c"))
    sm_sb = const.tile([128, 128], F32, tag="sm")
    nc.sync.dma_start(sm_sb[:], dram["sm"])
    sd4_sb = const.tile([128, 4 * 32], F16, tag="sd4")
    nc.sync.dma_start(sd4_sb[:].rearrange("p (d c) -> p d c", d=4),
                      dram["sd4"].rearrange("d p c -> p d c"))
    gc_sb = const.tile([128, 4], F32, tag="gc")
    nc.sync.dma_start(gc_sb[:], dram["gcols"])
    bc_sb = const.tile([128, 4], F32, tag="bc")
    nc.sync.dma_start(bc_sb[:], dram["bcols"])
    eps_sb = const.tile([128, 1], F32, tag="eps")
    nc.vector.memset(eps_sb[:], LN_EPS)

    wbd_v = wbd_sb[:].rearrange("p (q r c) -> p q r c", q=9, r=8)
    sn_v = sn_sb[:].rearrange("p (r c) -> p r c", r=8)
    sd4_v = sd4_sb[:].rearrange("p (d c) -> p d c", d=4)

    pending_tail = []

    for bi in range(nb):
        # split kl 0-4 / 5-8 so the next batch's first chunk can load as
        # soon as this batch's kl<5 votes are done (hides the reload)
        xuA = xpool.tile([128, 5 * 4 * HWO], F16, tag="xuA")
        nc.sync.dma_start(xuA[:], dram["xunf"][bi][:, :5 * 4 * HWO])
        xuB = xpool.tile([128, 4 * 4 * HWO], F16, tag="xuB")
        nc.sync.dma_start(xuB[:], dram["xunf"][bi][:, 5 * 4 * HWO:])
        xuA_v = xuA[:].rearrange("p (kl ah q) -> p kl ah q", kl=5, ah=2)
        xuB_v = xuB[:].rearrange("p (kl ah q) -> p kl ah q", kl=4, ah=2)

        def xu_v(kl, ah):
            return (xuA_v[:, kl, ah, :] if kl < 5
                    else xuB_v[:, kl - 5, ah, :])
        u_sb = upool.tile([128, 4 * 8 * HWO], F16, tag="u")
        for g in range(8):
            nc.sync.dma_start(u_sb[16 * g:16 * g + 16, :], dram["u_scr"][bi])

        # Software-pipelined over t = 4*kl + i (36 blocks/bi), 5 stages.
        T = 36
        st = {}

        def votes_mm(t, r, votes, drains):
            kl, i = divmod(t, 4)
            vt = vps.tile([128, 1024], F32, tag="vt", name=f"vt{t}_{r}")
            for ah in range(2):
                nc.tensor.matmul(
                    vt[:, 512 * ah:512 * ah + 450],
                    wbd_v[32 * i:32 * i + 32, kl, r, :],
                    xu_v(kl, ah)[32 * i:32 * i + 32, :],
                    start=True, stop=True,
                    tile_position=(32 * i, 0),
                )
            src = vt[:].rearrange("p (b q) -> p b q", b=2)[:, :, :450]\
                .rearrange("p b (a2 hw) -> p b a2 hw", a2=2)
            dst = votes[:].rearrange(
                "p (b a2 r hw) -> p b a2 r hw", b=2, a2=2, r=8)\
                [:, :, :, r, :]
            drains.append(('copy', dst, src))

        def emit_a(t):
            # pt = votes * u;  a-sum -> ptsum [128, (r8, hw)]
            votes = st['v', t]
            pt = ptp.tile([128, 4 * 8 * HWO], F16, tag="pt", name=f"pt{t}")
            nc.vector.tensor_mul(pt[:], votes[:], u_sb[:])
            s01 = psp.tile([128, 2 * 8 * HWO], F16, tag="s01",
                           name=f"s01_{t}")
            nc.vector.tensor_add(s01[:], pt[:][:, :2 * 8 * HWO],
                                 pt[:][:, 2 * 8 * HWO:])
            st['s01', t] = s01

        def sq_mm(t, half, exps):
            # logits (sum j via SQ, second a-fold via PSUM accum)
            s01 = st['s01', t]
            pv = s01[:].rearrange("p (a2 r hw) -> p a2 r hw", a2=2, r=8)
            e_sb = st['e', t]
            qk = qps.tile([128, 1024], F32, tag="qk",
                          name=f"qk{t}_{half}")
            for b2 in range(2):
                r2 = 2 * half + b2
                for a2 in range(2):
                    nc.tensor.matmul(
                        qk[:, 512 * b2:512 * b2 + 2 * HWO], sq_sb[:],
                        pv[:, a2, 2 * r2:2 * r2 + 2, :],
                        start=(a2 == 0), stop=(a2 == 1))
            dst = e_sb[:, half * 4 * HWO:(half + 1) * 4 * HWO]\
                .rearrange("p (b q) -> p b q", b=2)
            srcv = qk[:].rearrange("p (b q) -> p b q", b=2)[:, :, :2 * HWO]
            exps.append(('exp', dst, srcv))

        def emit_ers(t):
            # r-sum of e on DVE -> er (feeds next step's den matmul)
            e_sb = st['e', t]
            s1 = scr.tile([128, 4 * HWO], F16, tag="ers1", name=f"ers1_{t}")
            nc.vector.tensor_add(s1[:], e_sb[:][:, :4 * HWO],
                                 e_sb[:][:, 4 * HWO:])
            s2 = scr.tile([128, 2 * HWO], F16, tag="ers2", name=f"ers2_{t}")
            nc.vector.tensor_add(s2[:], s1[:][:, :2 * HWO],
                                 s1[:][:, 2 * HWO:])
            er = erp.tile([128, HWO], F16, tag="er", name=f"er{t}")
            nc.vector.tensor_add(er[:], s2[:][:, :HWO], s2[:][:, HWO:])
            st['er', t] = er

        def den_mm(t):
            er = st.pop(('er', t))
            den = qps.tile([128, 1024], F32, tag="qk", name=f"den{t}")
            nc.tensor.matmul(den[:, :HWO], sd_sb[:], er[:],
                             start=True, stop=True)
            st['den', t] = den

        def emit_qkt(t):
            # rd = 1/den; qkt = e * rd (r-bcast)
            e_sb = st.pop(('e', t))
            den = st.pop(('den', t))
            rd = rdp.tile([128, HWO], F32, tag="rd", name=f"rd{t}")
            nc.vector.reciprocal_approx_fast(out=rd[:], in_=den[:, :HWO])
            rdh = rdp.tile([128, HWO], F16, tag="rdh", name=f"rdh{t}")
            nc.vector.tensor_copy(rdh[:], rd[:])
            e_v = e_sb[:].rearrange("p (r hw) -> p r hw", r=8)
            qkt = qktp.tile([128, 8 * HWO], F16, tag="qkt", name=f"qkt{t}")
            rdb = rdh[:].rearrange("p (o hw) -> p o hw", o=1)\
                .broadcast_to((128, 8, HWO))
            nc.vector.tensor_mul(
                qkt[:].rearrange("p (r hw) -> p r hw", r=8), e_v, rdb)
            st['qkt', t] = qkt

        def emit_d(t):
            # p2 = votes * qkt (a-bcast); acc += p2 (gpsimd RMW-accum DMA)
            votes = st.pop(('v', t))
            qkt = st.pop(('qkt', t))
            qb = qkt[:].rearrange("p (o q) -> p o q", o=1)\
                .broadcast_to((128, 4, 8 * HWO))
            if t == 0:
                acc = pap.tile([128, 4 * 8 * HWO], F16, tag="pacc",
                               name=f"pacc{bi}")
                nc.vector.tensor_mul(
                    acc[:].rearrange("p (a q) -> p a q", a=4),
                    votes[:].rearrange("p (a q) -> p a q", a=4), qb)
                st['acc'] = acc
            else:
                p2 = p2p.tile([128, 4 * 8 * HWO], F16, tag="p2",
                              name=f"p2_{t}")
                nc.vector.tensor_mul(
                    p2[:].rearrange("p (a q) -> p a q", a=4),
                    votes[:].rearrange("p (a q) -> p a q", a=4), qb)
                acc = st['acc']
                for c in range(0, 4 * 8 * HWO, 8 * HWO):
                    nc.gpsimd.dma_start(
                        acc[:][:, c:c + 8 * HWO], p2[:][:, c:c + 8 * HWO],
                        accum_op=mybir.AluOpType.add)

        # Per step s the stages are: V(s) votes, A(s-1) pt/s01,
        # B(s-2) SQ+exp+ers, C(s-3) den/rd/qkt, D(s-4) p2/acc.
        # PE work is interleaved (den first, SQ between votes) so the
        # tensor engine streams without ring-stall gaps (HAM stays warm);
        # Act instructions are ordered by producer completion.
        for step in range(T + 4):
            if pending_tail:
                pending_tail.pop(0)()
            # --- PE head: den for t=s-3 (er was finished last step)
            if 3 <= step <= T + 2:
                den_mm(step - 3)
            # --- DVE: p2 first (inputs ready), then pt/s01 (ready), then
            # the rd/qkt chain (waits on this step's den matmul)
            if 4 <= step <= T + 3:
                emit_d(step - 4)
            if 1 <= step <= T:
                emit_a(step - 1)
            if 3 <= step <= T + 2:
                emit_qkt(step - 3)
            # --- PE stream (+ matching Act order)
            tb = step - 2 if 2 <= step <= T + 1 else None
            drains, exps = [], []
            if tb is not None:
                st['e', tb] = epool.tile([128, 8 * HWO], F16, tag="e",
                                         name=f"e{tb}")
            if tb is not None:
                sq_mm(tb, 0, exps)
                sq_mm(tb, 1, exps)
            if step < T:
                votes = vsb.tile([128, 4 * 8 * HWO], F16, tag="votes",
                                 name=f"votes{bi}_{step}")
                for r in range(8):
                    votes_mm(step, r, votes, drains)
                st['v', step] = votes
            # --- Act: exps first (feed this step's DVE r-sum), then drains
            order = exps + drains
            for kind, dst, src in order:
                if kind == 'exp':
                    nc.scalar.activation(
                        dst, src, mybir.ActivationFunctionType.Exp)
                else:
                    nc.scalar.copy(dst, src)
            # --- DVE tail of step: r-sum of this step's e
            if tb is not None:
                emit_ers(tb)

        pending_tail = _make_tail(tc, bi, st.pop('acc'), dram,
                                  consts=dict(sn_v=sn_v, sm_sb=sm_sb,
                                              sd4_v=sd4_v, gc_sb=gc_sb,
                                              bc_sb=bc_sb, eps_sb=eps_sb),
                                  pools=dict(mpool=mpool, spool=spool,
                                             qps=qps))

    for chunk in pending_tail:
        chunk()


def _make_tail(tc, bi, acc, dram, consts, pools):
    """LayerNorm + output permute for batch bi, split into chunks that are
    emitted interleaved with the next batch's pipeline steps."""
    nc = tc.nc
    sn_v, sm_sb, sd4_v = consts["sn_v"], consts["sm_sb"], consts["sd4_v"]
    gc_sb, bc_sb, eps_sb = consts["gc_sb"], consts["bc_sb"], consts["eps_sb"]
    mpool, spool, qps = pools["mpool"], pools["spool"], pools["qps"]
    env = {}

    def c0():
        # out-agg: 16 SN matmuls on p2acc; drain to SBUF right away
        acc_v = acc[:].rearrange("p (a r hw) -> p a r hw", a=4, r=8)
        oacc_t = qps.tile([128, 1024], F32, tag="qk", name=f"oacc{bi}")
        for r in range(8):
            for h in range(2):
                o0, onh = OHALVES[h]
                nc.tensor.matmul(
                    oacc_t[:, 512 * h:512 * h + 4 * onh], sn_v[:, r, :],
                    acc_v[:, :, r, o0:o0 + onh],
                    start=(r == 0), stop=(r == 7),
                )
        oaccS = spool.tile([128, 4 * HWO], F16, tag="oaccS")
        for h in range(2):
            o0, onh = OHALVES[h]
            nc.scalar.copy(
                oaccS[:, 4 * o0:4 * o0 + 4 * onh],
                oacc_t[:, 512 * h:512 * h + 4 * onh])
        env["oaccS"] = oaccS

    def c1():
        oaccS = env["oaccS"]
        s1 = mpool.tile([128, HWO], F32, tag="s1")
        s2 = mpool.tile([128, HWO], F32, tag="s2")
        sqt = spool.tile([128, 4 * HWO], F16, tag="outn",
                         name=f"sqt{bi}")
        for h in range(2):
            o0, onh = OHALVES[h]
            oa = oaccS[:, 4 * o0:4 * o0 + 4 * onh]
            nc.vector.tensor_reduce(
                s1[:, o0:o0 + onh],
                oa.rearrange("p (a q) -> p q a", a=4),
                axis=mybir.AxisListType.X, op=mybir.AluOpType.add)
            nc.scalar.activation(sqt[:, 4 * o0:4 * o0 + 4 * onh], oa,
                                 mybir.ActivationFunctionType.Square)
            nc.vector.tensor_reduce(
                s2[:, o0:o0 + onh],
                sqt[:, 4 * o0:4 * o0 + 4 * onh]
                .rearrange("p (a q) -> p q a", a=4),
                axis=mybir.AxisListType.X, op=mybir.AluOpType.add)
        mu_t = qps.tile([128, 1024], F32, tag="qk", name=f"mu{bi}")
        nc.tensor.matmul(mu_t[:, :HWO], sm_sb[:], s1[:],
                         start=True, stop=True)
        nc.tensor.matmul(mu_t[:, 512:512 + HWO], sm_sb[:], s2[:],
                         start=True, stop=True)
        mu = mpool.tile([128, HWO], F32, tag="mu")
        nc.scalar.copy(mu[:], mu_t[:, :HWO])
        # reuse s1/s2 buffers: musq -> s1, var -> s2
        nc.vector.tensor_mul(s1[:], mu[:], mu[:])
        nc.vector.tensor_sub(s2[:], mu_t[:, 512:512 + HWO], s1[:])
        env["mu"], env["s1"], env["var"] = mu, s1, s2

    def c2():
        mu, var, sig = env["mu"], env["var"], env["s1"]
        oaccS = env["oaccS"]
        nc.scalar.activation(sig[:], var[:],
                             mybir.ActivationFunctionType.Sqrt,
                             bias=eps_sb[:, 0:1])
        rstd = var  # reuse
        nc.vector.reciprocal(rstd[:], sig[:])
        outn = spool.tile([128, 4 * HWO], F16, tag="outn")  # (a, hw)
        t3 = mpool.tile([128, HWO], F32, tag="t3")
        for h in range(2):
            o0, onh = OHALVES[h]
            for a in range(4):
                nc.vector.tensor_sub(
                    t3[:, :onh],
                    oaccS[:, 4 * o0 + a * onh:4 * o0 + (a + 1) * onh],
                    mu[:, o0:o0 + onh])
                nc.vector.tensor_mul(t3[:, :onh], t3[:, :onh],
                                     rstd[:, o0:o0 + onh])
                nc.vector.tensor_scalar(
                    outn[:, a * HWO + o0:a * HWO + o0 + onh],
                    t3[:, :onh], gc_sb[:, a:a + 1], bc_sb[:, a:a + 1],
                    op0=mybir.AluOpType.mult, op1=mybir.AluOpType.add)
        env["outn"] = outn

    def c3():
        outn = env["outn"]
        outn_v = outn[:].rearrange("p (a hw) -> p a hw", a=4)
        out_v = dram["out"][bi].rearrange("p (hw ad) -> p hw ad", ad=16)
        for h in range(2):
            o0, onh = OHALVES[h]
            fin = spool.tile([32, 113 * 16], F16, tag="fin",
                             name=f"fin{bi}_{h}")
            fin_v = fin[:][:, :onh * 16].rearrange(
                "p (hw a x) -> p a hw x", a=4, x=4)
            for d in range(4):
                fp_t = qps.tile([128, 1024], F32, tag="qk",
                                name=f"fp{bi}_{h}_{d}")
                fp = fp_t[:32, :4 * onh]
                nc.tensor.matmul(
                    fp, sd4_v[:, d, :],
                    outn_v[:, :, o0:o0 + onh],
                    start=True, stop=True)
                nc.scalar.copy(fin_v[:, :, :, d],
                               fp.rearrange("p (a q) -> p a q", a=4))
            nc.sync.dma_start(out_v[:, o0:o0 + onh, :],
                              fin[:][:, :onh * 16])

    return [c0, c1, c2, c3]


# ------------------------------------------------------------------ driver
def _build_nc(nb=NB):
    nc = bacc.Bacc("TRN2", target_bir_lowering=False, debug=False,
                   num_devices=NCORES)
    dram = {}
    dram["xunf"] = nc.dram_tensor("xunf", (nb, 128, 9 * 4 * HWO), F16,
                                  kind="ExternalInput").ap()
    dram["u_scr"] = nc.dram_tensor("u_scr", (nb, 16, 4 * 8 * HWO), F16,
                                   kind="ExternalInput").ap()
    dram["w_c"] = nc.dram_tensor("w_c", (9, 8, 128, 16), F16,
                                 kind="ExternalInput").ap()
    dram["gmask"] = nc.dram_tensor("gmask", (128, 128), F16,
                                   kind="ExternalInput").ap()
    dram["sq"] = nc.dram_tensor("sq", (128, 128), F16,
                                kind="ExternalInput").ap()
    dram["sd"] = nc.dram_tensor("sd", (128, 128), F16,
                                kind="ExternalInput").ap()
    dram["sn"] = nc.dram_tensor("sn", (8, 128, 128), F16,
                                kind="ExternalInput").ap()
    dram["sm"] = nc.dram_tensor("sm", (128, 128), F32,
                                kind="ExternalInput").ap()
    dram["sd4"] = nc.dram_tensor("sd4", (4, 128, 32), F16,
                                 kind="ExternalInput").ap()
    dram["gcols"] = nc.dram_tensor("gcols", (128, 4), F32,
                                   kind="ExternalInput").ap()
    dram["bcols"] = nc.dram_tensor("bcols", (128, 4), F32,
                                   kind="ExternalInput").ap()
    dram["out"] = nc.dram_tensor("out", (nb, 32, HWO * 16), F16,
                                 kind="ExternalOutput").ap()
    with tile.TileContext(nc) as tc:
        build_program(tc, dram, nb=nb)
    nc.compile()
    return nc


def _make_in_maps(hp):
    shared = {k: hp[k] for k in
              ("w_c", "gmask", "sq", "sd", "sn", "sm", "sd4",
               "gcols", "bcols")}
    in_maps = []
    for c in range(NCORES):
        im = dict(shared)
        im["xunf"] = np.ascontiguousarray(hp["xunf"][c * NB:(c + 1) * NB])
        im["u_scr"] = np.ascontiguousarray(hp["u_scr"][c * NB:(c + 1) * NB])
        in_maps.append(im)
    return in_maps


def _prep(**inputs):
    input_ = np.asarray(inputs["input"], dtype=np.float32)
    ncv = np.asarray(inputs["next_capsule_value"], dtype=np.float32)
    w = np.asarray(inputs["w"], dtype=np.float32)
    gamma = np.asarray(inputs["gamma"], dtype=np.float32)
    beta = np.asarray(inputs["beta"], dtype=np.float32)
    return host_prep(input_, ncv, w, gamma, beta)


def _to_full(outs):
    full = np.concatenate([np.asarray(o["out"]) for o in outs], axis=0)
    return full.reshape(B, M, HO, WO, DOUT).astype(np.float32)


def kernel(**inputs):
    hp = _prep(**inputs)
    nc = _build_nc()
    in_maps = _make_in_maps(hp)
    from concourse.bass_utils import run_bass_kernel_spmd
    res = run_bass_kernel_spmd(nc, in_maps, core_ids=list(range(NCORES)),
                               trace=False)
    return _to_full(res.results)


def kernel_traced(**inputs):
    hp = _prep(**inputs)
    nc = _build_nc()
    in_maps = _make_in_maps(hp)
    from concourse.bass_utils import run_bass_kernel_spmd
    res = run_bass_kernel_spmd(nc, in_maps, core_ids=list(range(NCORES)),
                               trace=True)
    return _to_full(res.results), res


def kernel_bench(reps=30, **inputs):
    """Correctness output + repeated-call wall times (ns) of the compiled
    8-core executable (device-resident inputs)."""
    import time
    import jax
    from jax.experimental.shard_map import shard_map
    from jax.sharding import Mesh, PartitionSpec
    from concourse import bass2jax, mybir as _mb

    hp = _prep(**inputs)
    nc = _build_nc()
    in_maps = _make_in_maps(hp)

    bass2jax.install_neuronx_cc_hook()
    partition_name = (nc.partition_id_tensor.name
                      if nc.partition_id_tensor else None)
    in_names, out_names, out_avals, zero_outs = [], [], [], []
    for alloc in nc.m.functions[0].allocations:
        if not isinstance(alloc, _mb.MemoryLocationSet):
            continue
        name = alloc.memorylocations[0].name
        if alloc.kind == "ExternalInput":
            if name != partition_name:
                in_names.append(name)
        elif alloc.kind == "ExternalOutput":
            shape = tuple(alloc.tensor_shape)
            dtype = _mb.dt.np(alloc.dtype)
            out_names.append(name)
            out_avals.append(jax.core.ShapedArray(shape, dtype))
            zero_outs.append(np.zeros(shape, dtype))
    n_params = len(in_names)
    all_in_names = list(in_names) + list(out_names)
    if partition_name is not None:
        all_in_names.append(partition_name)

    def _body(*args):
        operands = list(args)
        if partition_name is not None:
            operands.append(bass2jax.partition_id_tensor())
        outs = bass2jax._bass_exec_p.bind(
            *operands, out_avals=tuple(out_avals),
            in_names=tuple(all_in_names), out_names=tuple(out_names),
            lowering_input_output_aliases=(),
            sim_require_finite=True, sim_require_nnan=True, nc=nc)
        return tuple(outs)

    devices = jax.devices()[:NCORES]
    mesh = Mesh(np.asarray(devices), ("core",))
    n_outs = len(out_names)
    sharded = jax.jit(
        shard_map(_body, mesh=mesh,
                  in_specs=(PartitionSpec("core"),) * (n_params + n_outs),
                  out_specs=(PartitionSpec("core"),) * n_outs,
                  check_rep=False),
        keep_unused=True)
    concat_in = [np.concatenate([np.asarray(in_maps[c][nm])
                                 for c in range(NCORES)], axis=0)
                 for nm in in_names]
    concat_zeros = [np.zeros((NCORES * z.shape[0], *z.shape[1:]), z.dtype)
                    for z in zero_outs]
    dev_in = [jax.device_put(a) for a in concat_in]
    dev_zero = [jax.device_put(a) for a in concat_zeros]
    outs = sharded(*dev_in, *dev_zero)
    jax.block_until_ready(outs)
    ts = []
    for _ in range(reps):
        t0 = time.perf_counter()
        o = sharded(*dev_in, *dev_zero)
        jax.block_until_ready(o)
        ts.append(time.perf_counter() - t0)
    out_full = np.asarray(outs[out_names.index("out")]).reshape(
        NCORES, NB, 32, HWO * 16)
    full = out_full.reshape(B, M, HO, WO, DOUT).astype(np.float32)
    return full, np.array(ts) * 1e9


if __name__ == "__main__":
    nc = _build_nc()
    print("built OK")
    from concourse.timeline_sim import TimelineSim
    ts = TimelineSim(nc, no_exec=True)
    print("TimelineSim duration ns:", ts.simulate())
